# revision 37
# baseline (speedup 1.0000x reference)
"""Longhorn SSM layer on 8 Trainium2 cores.

Sharding: core (b, j) with b in {0,1}, j in {0..3} handles batch b and
d_inner channel chunk [j*512, (j+1)*512).  The x_proj contraction needs all
d_inner channels, so partial x_dbl results are AllReduced across the 4 cores
of each batch (two half-L collectives).  out_proj partials sum on the host.

v4 notes (vs v2 ~704us, v3 ~893us):
  - DVE ops batched over n-PAIRS at full 2048-element size (the v3 half-L
    split cost ~+130us because per-instruction DVE overhead is ~200-500
    cycles, not the nominal 58): c/a/b/s/p tiles are [128, 2, 1024], the
    scan runs on the flattened [128, 2048] view with the pair boundary
    handled by forcing a=0 at the segment start (h0) or folding the carry
    into b via scalar_tensor_tensor (h1) -- both validated on HW
  - phase B starts on the h0 half right after the first collective (~100us)
    via half-L scans with carry chaining; v2 waited for both halves (~180us)
  - broadcasts hoisted: each kk/k/q n-pair broadcast tile [128,2,1024] is
    shared by a g-PAIR (main g0/g1, then trail g2/g3)
  - ACT table thrash fixed: dtv chains grouped ([0,1],[2,3]), carry copies
    are Identity activations (same table as the a-pass)
  - out_proj: its own 2-buf PSUM pool (opened after the phase-A pools
    close), pieces emitted as soon as their ygb inputs exist so they hide
    under later scan work; only (g2,g3)-h1 lands in the tail
  - z gate staged through DRAM (zbd) and re-loaded per (g,h) at drain time,
    freeing SBUF for the batched scan tiles
  - all matmuls bf16; scan a-operand fp16 (scan is 2cyc/elem regardless of
    dtype -- measured -- but fp16 halves SBUF vs f32 and keeps precision)
"""

import numpy as np
import ml_dtypes

import concourse.bacc as bacc
import concourse.bass as bass
import concourse.tile as tile
from concourse import mybir
from concourse.bass_utils import run_bass_kernel_spmd

F32 = mybir.dt.float32
BF16 = mybir.dt.bfloat16
F16 = mybir.dt.float16
AL = mybir.AluOpType
AF = mybir.ActivationFunctionType

BF = ml_dtypes.bfloat16


def build_module(L, DM, DI, DCH, NST, DTR, num_devices, use_collective):
    NG = DCH // 128          # d-tiles per core (4)
    NK = DM // 128           # K-tiles for in_proj (8)
    NO = DM // 128           # out_proj output tiles (8)
    TQ = 512                 # matmul moving-dim tile
    NTQ = L // TQ            # 4
    LH = L // 2              # half length (1024)
    NP = NST // 2            # n-pairs (8)
    NR = DTR + 2 * NST       # x_proj rows (96)
    PAD = 3                  # conv left pad

    nc = bacc.Bacc(
        "TRN2",
        target_bir_lowering=False,
        debug=False,
        enable_asserts=False,
        num_devices=num_devices,
    )

    # ---- I/O -------------------------------------------------------------
    hT_d = nc.dram_tensor("hT", [DM, L], BF16, kind="ExternalInput")
    wx_d = nc.dram_tensor("wx", [128, NK * NG * 128], BF16, kind="ExternalInput")
    wz_d = nc.dram_tensor("wz", [128, NK * NG * 128], BF16, kind="ExternalInput")
    wo_d = nc.dram_tensor("wo", [128, NG * NO * 128], BF16, kind="ExternalInput")
    dtw_d = nc.dram_tensor("dtw", [DTR, NG * 128], BF16, kind="ExternalInput")
    xpw_d = nc.dram_tensor("xpw", [128, NG * NR], BF16, kind="ExternalInput")
    cwd_d = nc.dram_tensor("cwd", [128, NG * 4 * 128], BF16, kind="ExternalInput")
    dgd_d = nc.dram_tensor("dgd", [128, NG * 128], BF16, kind="ExternalInput")
    pvec_d = nc.dram_tensor("pvec", [128, NG * 2], F32, kind="ExternalInput")
    ones_d = nc.dram_tensor("ones16", [NST, 128], BF16, kind="ExternalInput")
    id_d = nc.dram_tensor("id128", [128, 128], BF16, kind="ExternalInput")
    outA_d = nc.dram_tensor("outA", [DM, L], F32, kind="ExternalOutput")
    outB_d = nc.dram_tensor("outB", [DM, L], BF16, kind="ExternalOutput")

    # internal DRAM
    cc_in = [nc.dram_tensor(f"ccin{q}", [NR, LH], BF16, kind="Internal")
             for q in range(2)]
    cc_out = [nc.dram_tensor(f"ccout{q}", [NR, LH], BF16, kind="Internal")
              for q in range(2)]
    kbd = nc.dram_tensor("kbd", [NST, L], BF16, kind="Internal")
    qbd = nc.dram_tensor("qbd", [NST, L], BF16, kind="Internal")
    kkbd = nc.dram_tensor("kkbd", [NST, L], BF16, kind="Internal")
    zbd = nc.dram_tensor("zbd", [128, NG * L], BF16, kind="Internal")

    groups = [[0, 1, 2, 3], [4, 5, 6, 7]] if num_devices == 8 else [[0]]

    with tile.TileContext(nc) as tc:
        with (
            tc.tile_pool(name="const", bufs=1) as constp,
            tc.tile_pool(name="persist", bufs=1) as pp,
            # PSUM reserved up front: 2x2-bank Y slots + 1 bank for psK/psD
            tc.tile_pool(name="psY", bufs=2, space="PSUM") as psYp,
            tc.tile_pool(name="pq", bufs=2, space="PSUM") as pqp,
            # pools the h0 pipeline writes from ~75us on -- kept OUT of the
            # phase-A region so region-reuse WAR edges never delay them
            tc.tile_pool(name="rows", bufs=1) as rowp,
            tc.tile_pool(name="dtv", bufs=1) as dtvp,
            tc.tile_pool(name="bcast", bufs=2) as bcp,
            tc.tile_pool(name="scan", bufs=1) as scp,
            tc.tile_pool(name="zjit", bufs=2) as zjp,
        ):
            ones_sb = constp.tile([NST, 128], BF16)
            nc.sync.dma_start(ones_sb, ones_d.ap())
            id_sb = constp.tile([128, 128], BF16)
            nc.sync.dma_start(id_sb, id_d.ap())
            dgd_sb = constp.tile([128, NG, 128], BF16)
            nc.sync.dma_start(dgd_sb, dgd_d.ap().rearrange("p (g m) -> p g m", g=NG))
            pvec = constp.tile([128, NG, 2], F32)   # [...,0]=-dtb, [...,1]=conv_b
            nc.sync.dma_start(pvec, pvec_d.ap().rearrange("p (g c) -> p g c", g=NG))
            dtw_sb = constp.tile([DTR, NG, 128], BF16)

            # persistent SBUF, split per L-half so h0 readers never wait on
            # h1 writers; ygb split by out_proj piece
            xs = [pp.tile([128, NG, LH], BF16, name=f"xs{h}") for h in range(2)]
            dtvb = [pp.tile([128, NG, LH], BF16, name=f"dtvb{h}") for h in range(2)]
            ubt = [pp.tile([128, NG, LH], BF16, name=f"ub{h}") for h in range(2)]
            ygAB = [pp.tile([128, 2, LH], BF16, name=f"ygAB{h}") for h in range(2)]
            ygC = [pp.tile([128, LH], BF16, name=f"ygC{h}") for h in range(2)]
            yg3 = [pp.tile([128, LH], BF16, name=f"yg3{h}") for h in range(2)]
            carry = pp.tile([128, NG, NST], F32, name="carry")

            # ---------------- phase A: in_proj/conv/x_dbl + CC ------------
            with (
                tc.tile_pool(name="hw", bufs=1) as hwp,
                tc.tile_pool(name="xpre", bufs=1) as xprep,
                tc.tile_pool(name="psA", bufs=2, space="PSUM") as psA,
                tc.tile_pool(name="asm", bufs=2) as asmp,
            ):
                # DMA priority: first-quarter activations + x-weights first
                wx_sb = hwp.tile([128, NK, NG, 128], BF16)
                hsbs = []
                for tq in range(NTQ):
                    hsbs.append(hwp.tile([128, NK, TQ], BF16, name=f"hsb{tq}",
                                         tag="hsb", bufs=2))
                for k in range(NK):
                    nc.sync.dma_start(
                        hsbs[0][:, k], hT_d.ap()[k * 128:(k + 1) * 128, 0:TQ])
                    nc.sync.dma_start(
                        wx_sb[:, k], wx_d.ap()[:, k * NG * 128:(k + 1) * NG * 128]
                        .rearrange("p (g m) -> p g m", g=NG))
                cw_sb = hwp.tile([128, NG, 4, 128], BF16)
                nc.sync.dma_start(
                    cw_sb, cwd_d.ap().rearrange("p (g j m) -> p g j m", g=NG, j=4))
                xpw_sb = hwp.tile([128, NG, NR], BF16)
                for g in range(NG):
                    nc.sync.dma_start(
                        xpw_sb[:, g], xpw_d.ap()[:, g * NR:(g + 1) * NR])
                for tq in range(1, NTQ):
                    ts = slice(tq * TQ, (tq + 1) * TQ)
                    for k in range(NK):
                        nc.sync.dma_start(
                            hsbs[tq][:, k], hT_d.ap()[k * 128:(k + 1) * 128, ts])
                wz_sb = hwp.tile([128, NK, NG, 128], BF16)
                for k in range(NK):
                    nc.sync.dma_start(
                        wz_sb[:, k], wz_d.ap()[:, k * NG * 128:(k + 1) * NG * 128]
                        .rearrange("p (g m) -> p g m", g=NG))
                nc.sync.dma_start(
                    dtw_sb, dtw_d.ap().rearrange("p (g m) -> p g m", g=NG))

                # rolling conv window: xpre_q = [3-col tail of prev | 512 new]
                xpres = []
                for tq in range(NTQ):
                    xpres.append(xprep.tile([128, NG, TQ + PAD], BF16,
                                            name=f"xpre{tq}", tag="xpre",
                                            bufs=2))
                for g in range(NG):
                    nc.vector.memset(xpres[0][:, g, 0:PAD], 0.0)

                def emit_z_quarter(tq):
                    h, q2 = divmod(tq, 2)
                    for g in range(NG):
                        psz = psA.tile([128, TQ], F32, name="psz", tag="psA")
                        for k in range(NK):
                            nc.tensor.matmul(psz, wz_sb[:, k, g, :],
                                             hsbs[tq][:, k, :],
                                             start=(k == 0), stop=(k == NK - 1))
                        # store z PRE-activation (DVE copy): keeps the Silu
                        # off the ACT critical window; silu applied at drain
                        zt = asmp.tile([128, TQ], BF16, name="ztq", tag="ztq")
                        nc.vector.tensor_copy(zt, psz)
                        nc.sync.dma_start(
                            zbd.ap()[:, g * L + tq * TQ: g * L + (tq + 1) * TQ],
                            zt)

                # x-side per quarter: in_proj + conv; half-CC after q1/q3
                for tq in range(NTQ):
                    h, q2 = divmod(tq, 2)
                    hts = slice(q2 * TQ, (q2 + 1) * TQ)
                    for g in range(NG):
                        ps = psA.tile([128, TQ], F32, name="ps_x", tag="psA")
                        for k in range(NK):
                            nc.tensor.matmul(ps, wx_sb[:, k, g, :],
                                             hsbs[tq][:, k, :],
                                             start=(k == 0), stop=(k == NK - 1))
                        # DVE copies: keeps ACT on Silu only (no act-table
                        # thrash) and DVE is idle through phase A anyway
                        nc.vector.tensor_copy(xpres[tq][:, g, PAD:PAD + TQ],
                                              ps)
                        if tq + 1 < NTQ:
                            nc.vector.tensor_copy(xpres[tq + 1][:, g, 0:PAD],
                                                  ps[:, TQ - PAD:TQ])
                        pc = psA.tile([128, TQ], F32, name="pc", tag="psA")
                        for j in range(4):
                            nc.tensor.matmul(
                                pc, cw_sb[:, g, j, :],
                                xpres[tq][:, g, j:j + TQ],
                                start=(j == 0), stop=(j == 3))
                        nc.scalar.activation(xs[h][:, g, hts], pc, AF.Silu,
                                             bias=pvec[:, g, 1:2])
                    psX = psA.tile([NR, TQ], F32, name="psX", tag="psA")
                    for g in range(NG):
                        nc.tensor.matmul(psX, xpw_sb[:, g, :], xs[h][:, g, hts],
                                         start=(g == 0), stop=(g == NG - 1))
                    xdp = asmp.tile([NR, TQ], BF16, name="xdp", tag="xdp")
                    nc.vector.tensor_copy(xdp, psX)
                    nc.sync.dma_start(
                        cc_in[h].ap()[:, q2 * TQ:(q2 + 1) * TQ], xdp)
                    if q2 == 1:
                        if use_collective:
                            nc.gpsimd.collective_compute(
                                "AllReduce", AL.add, replica_groups=groups,
                                ins=[cc_in[h].ap()], outs=[cc_out[h].ap()])
                        else:
                            nc.sync.dma_start(cc_out[h].ap(), cc_in[h].ap())
                    # z-quarters 0/1 fill the PE idle under the h0 CC's wire
                    # time (and free hsb slots 0/1 for hsb2/hsb3 at bufs=2);
                    # z2/z3 follow x3 so their drains clear the ACT before
                    # the A2-h0 dtv chain starts
                    if tq == 1:
                        emit_z_quarter(0)
                        emit_z_quarter(1)
                emit_z_quarter(2)
                emit_z_quarter(3)

            # phase-A pools closed: out_proj + output-drain pools may reuse
            # that SBUF/PSUM region (their first use is far later)
            with (
                tc.tile_pool(name="wo", bufs=1) as wop,
                tc.tile_pool(name="odr", bufs=1) as odp,
                tc.tile_pool(name="po2", bufs=2, space="PSUM") as po2p,
            ):
                wo_sb = wop.tile([128, NG, NO, 128], BF16)

                # ---- A2: dt/k/q rows + dtv chain for one half ------------
                def emit_A2(h):
                    hs = slice(h * LH, (h + 1) * LH)
                    dtl = rowp.tile([DTR, LH], BF16, name="dtl", tag="dtl",
                                    bufs=1)
                    krow = rowp.tile([NST, LH], BF16, name="krow", tag="krow",
                                     bufs=1)
                    src = cc_out[h].ap()
                    nc.sync.dma_start(krow, src[DTR:DTR + NST, :])
                    nc.sync.dma_start(dtl, src[0:DTR, :])
                    kkb16 = rowp.tile([NST, LH], BF16, name="kkb16",
                                      tag="kkb16", bufs=1)
                    nc.scalar.activation(kkb16, krow, AF.Square)
                    nc.sync.dma_start(kkbd.ap()[:, hs], kkb16)
                    nc.sync.dma_start(kbd.ap()[:, hs], src[DTR:DTR + NST, :])
                    nc.sync.dma_start(qbd.ap()[:, hs], src[DTR + NST:NR, :])
                    # SK[t] = sum_n kk broadcast to 128 partitions
                    psKs = rowp.tile([128, LH], BF16, name="psKs", tag="psKs",
                                     bufs=1)
                    for s2 in range(2):
                        ss = slice(s2 * TQ, (s2 + 1) * TQ)
                        psK = pqp.tile([128, TQ], F32, name="psK", tag="pq")
                        nc.tensor.matmul(psK, ones_sb, kkb16[:, ss],
                                         start=True, stop=True)
                        nc.vector.tensor_copy(psKs[:, ss], psK)

                    # dtv = 1/(1+E+SK) = sigmoid(-ln(E+SK)); E=exp(-(dt+dtb))
                    def dtv_chain(gl):
                        egs, dens, lnws = [], [], []
                        for g in gl:
                            eg = dtvp.tile([128, LH], BF16, name=f"eg{g}",
                                           tag="eg", bufs=2)
                            for s2 in range(2):
                                ss = slice(s2 * TQ, (s2 + 1) * TQ)
                                psD = pqp.tile([128, TQ], F32, name="psD",
                                               tag="pq")
                                nc.tensor.matmul(psD, dtw_sb[:, g, :],
                                                 dtl[:, ss], start=True,
                                                 stop=True)
                                nc.scalar.activation(eg[:, ss], psD, AF.Exp,
                                                     bias=pvec[:, g, 0:1],
                                                     scale=-1.0)
                            egs.append(eg)
                        for g, eg in zip(gl, egs):
                            den = dtvp.tile([128, LH], BF16, name=f"den{g}",
                                            tag="den", bufs=2)
                            nc.vector.tensor_tensor(den, eg, psKs, op=AL.add)
                            dens.append(den)
                        for g, den in zip(gl, dens):
                            lnw = dtvp.tile([128, LH], BF16, name=f"lnw{g}",
                                            tag="lnw", bufs=2)
                            nc.scalar.activation(lnw, den, AF.Ln)
                            lnws.append(lnw)
                        for g, lnw in zip(gl, lnws):
                            nc.scalar.activation(dtvb[h][:, g, :], lnw,
                                                 AF.Sigmoid, scale=-1.0)
                            nc.vector.tensor_tensor(ubt[h][:, g, :],
                                                    xs[h][:, g, :],
                                                    dtvb[h][:, g, :],
                                                    op=AL.mult)

                    dtv_chain([0, 1])
                    dtv_chain([2, 3])

                # ---- phase B: n-pair batched scans for a g-pair ----------
                # All tiles are FLAT [128, 2*LH] (a rearranged 3-dim AP pays
                # a per-AP-row init inside DVE/ACT instructions: measured
                # scan 5202 vs 4336, ACT 2334 vs 1893).  Only the c/b mults
                # use 3-dim views, forced by the stride-0 dtv/ub broadcast.
                L2 = 2 * LH

                def scan_pair(gl, h):
                    hs = slice(h * LH, (h + 1) * LH)
                    for np_ in range(NP):
                        n0 = 2 * np_
                        bts = {}
                        for nm, dram in (("kkb", kkbd), ("kb", kbd),
                                         ("qb", qbd)):
                            t = bcp.tile([128, L2], BF16, name=nm, tag=nm)
                            for s in range(2):
                                nc.sync.dma_start(
                                    t[:, s * LH:(s + 1) * LH],
                                    dram.ap()[n0 + s:n0 + s + 1, hs]
                                    .broadcast_to([128, LH]))
                            bts[nm] = t
                        for g in gl:
                            c2 = scp.tile([128, L2], BF16, name="c2",
                                          tag="c", bufs=1)
                            nc.vector.tensor_tensor(
                                c2.rearrange("p (a b) -> p a b", a=2),
                                dtvb[h][:, g, :].unsqueeze(1)
                                .broadcast_to([128, 2, LH]),
                                bts["kkb"].rearrange("p (a b) -> p a b", a=2),
                                op=AL.mult)
                            a2 = scp.tile([128, L2], F16, name="a2",
                                          tag="a", bufs=2)
                            nc.scalar.activation(a2, c2, AF.Identity,
                                                 bias=1.0, scale=-1.0)
                            b2 = scp.tile([128, L2], BF16, name="b2",
                                          tag="b", bufs=1)
                            nc.vector.tensor_tensor(
                                b2.rearrange("p (a b) -> p a b", a=2),
                                ubt[h][:, g, :].unsqueeze(1)
                                .broadcast_to([128, 2, LH]),
                                bts["kb"].rearrange("p (a b) -> p a b", a=2),
                                op=AL.mult)
                            if h == 0:
                                init = 0.0
                            else:
                                # fold the second segment's carry into b,
                                # then reset a at the boundary
                                nc.vector.scalar_tensor_tensor(
                                    b2[:, LH:LH + 1], a2[:, LH:LH + 1],
                                    carry[:, g, n0 + 1:n0 + 2],
                                    b2[:, LH:LH + 1],
                                    op0=AL.mult, op1=AL.add)
                                init = carry[:, g, n0:n0 + 1]
                            nc.vector.memset(a2[:, LH:LH + 1], 0.0)
                            s2 = scp.tile([128, L2], BF16, name="s2",
                                          tag="s", bufs=1)
                            nc.vector.tensor_tensor_scan(
                                s2, a2, b2, init, op0=AL.mult, op1=AL.add)
                            if h == 0:
                                for s in range(2):
                                    nc.scalar.activation(
                                        carry[:, g, n0 + s:n0 + s + 1],
                                        s2[:, (s + 1) * LH - 1:(s + 1) * LH],
                                        AF.Identity)
                            p2 = scp.tile([128, L2], BF16, name="p2",
                                          tag="p", bufs=2)
                            nc.vector.tensor_tensor(p2, s2, bts["qb"],
                                                    op=AL.mult)
                            for s in range(2):
                                for c2_ in range(2):
                                    cs = slice(s * LH + c2_ * TQ,
                                               s * LH + (c2_ + 1) * TQ)
                                    ys = slice(c2_ * TQ, (c2_ + 1) * TQ)
                                    nc.tensor.matmul(
                                        Y[g][:, ys], id_sb, p2[:, cs],
                                        start=(np_ == 0 and s == 0),
                                        stop=False)

                def finish_g(g, h):
                    zt = zjp.tile([128, LH], BF16, name="zjit", tag="zjit")
                    nc.sync.dma_start(
                        zt, zbd.ap()[:, g * L + h * LH: g * L + (h + 1) * LH])
                    zsil = zjp.tile([128, LH], BF16, name="zsil", tag="zsil")
                    nc.scalar.activation(zsil, zt, AF.Silu)
                    for c2_ in range(2):
                        cs = slice(c2_ * TQ, (c2_ + 1) * TQ)
                        nc.tensor.matmul(
                            Y[g][:, cs], dgd_sb[:, g, :], xs[h][:, g, cs],
                            start=False, stop=True)
                    if g < 2:
                        dst = ygAB[h][:, g, :]
                    elif g == 2:
                        dst = ygC[h]
                    else:
                        dst = yg3[h]
                    nc.vector.tensor_tensor(dst, Y[g], zsil, op=AL.mult)

                # ---- out_proj pieces: chain over the given ygb tiles -----
                def emit_oproj(h, parts, dest, dt, drains):
                    # parts: list of (tile, extra-dim index or None)
                    hs = slice(h * LH, (h + 1) * LH)
                    for o in range(NO):
                        ot = odp.tile([128, LH], dt, name=f"ot{dt}",
                                      tag=f"ot{dt}", bufs=2)
                        for tq2 in range(2):
                            ss = slice(tq2 * TQ, (tq2 + 1) * TQ)
                            po = po2p.tile([128, TQ], F32, name="po",
                                           tag="po")
                            for i, (yt, gg, gi) in enumerate(parts):
                                src = yt[:, gi, ss] if gi is not None \
                                    else yt[:, ss]
                                nc.tensor.matmul(po, wo_sb[:, gg, o, :], src,
                                                 start=(i == 0),
                                                 stop=(i == len(parts) - 1))
                            if drains == "act" or tq2 % 2 == 0:
                                nc.scalar.copy(ot[:, ss], po)
                            else:
                                nc.vector.tensor_copy(ot[:, ss], po)
                        nc.sync.dma_start(
                            dest.ap()[o * 128:(o + 1) * 128, hs], ot)

                # ---------------- emission sequence -----------------------
                emit_A2(0)
                Y = {}
                Y[0] = psYp.tile([128, LH], F32, name="Y0", tag="Y")
                Y[1] = psYp.tile([128, LH], F32, name="Y1", tag="Y")
                scan_pair([0, 1], 0)
                # wo load kept off the DMA queues until after the first
                # scan block's broadcasts are queued (needed only ~450us)
                for g2 in range(NG):
                    nc.sync.dma_start(
                        wo_sb[:, g2],
                        wo_d.ap()[:, g2 * NO * 128:(g2 + 1) * NO * 128]
                        .rearrange("p (o m) -> p o m", o=NO))
                finish_g(0, 0)
                finish_g(1, 0)
                Y[2] = psYp.tile([128, LH], F32, name="Y2", tag="Y")
                Y[3] = psYp.tile([128, LH], F32, name="Y3", tag="Y")
                scan_pair([2, 3], 0)
                finish_g(2, 0)
                finish_g(3, 0)

                emit_A2(1)
                # h0 out_proj: runs during h1 scans (ACT drains keep DVE free)
                emit_oproj(0, [(ygAB[0], 0, 0), (ygAB[0], 1, 1),
                               (ygC[0], 2, None)], outA_d, F32, "act")
                emit_oproj(0, [(yg3[0], 3, None)], outB_d, BF16, "act")

                Y[0] = psYp.tile([128, LH], F32, name="Y0b", tag="Y")
                Y[1] = psYp.tile([128, LH], F32, name="Y1b", tag="Y")
                scan_pair([0, 1], 1)
                finish_g(0, 1)
                finish_g(1, 1)
                # (g0,g1)-h1 out_proj: runs during the (g2,g3)-h1 scans
                emit_oproj(1, [(ygAB[1], 0, 0), (ygAB[1], 1, 1)],
                           outA_d, F32, "act")
                Y[2] = psYp.tile([128, LH], F32, name="Y2b", tag="Y")
                Y[3] = psYp.tile([128, LH], F32, name="Y3b", tag="Y")
                scan_pair([2, 3], 1)
                finish_g(2, 1)
                finish_g(3, 1)
                # tail: (g2,g3)-h1, mixed DVE/ACT drains
                emit_oproj(1, [(ygC[1], 2, None), (yg3[1], 3, None)],
                           outB_d, BF16, "mixed")

    nc.compile()
    return nc


# ----------------------------------------------------------------------------
# host-side packing
# ----------------------------------------------------------------------------

def pack_core_inputs(inputs, b, j, L, DM, DI, DCH, NST, DTR):
    NG = DCH // 128
    NK = DM // 128
    NO = DM // 128
    NR = DTR + 2 * NST
    ch = slice(j * DCH, (j + 1) * DCH)

    h = np.asarray(inputs["hidden_states"], np.float32)
    ipw = np.asarray(inputs["in_proj_w"], np.float32)
    cw = np.asarray(inputs["conv_w"], np.float32).reshape(DI, 4)
    cb = np.asarray(inputs["conv_b"], np.float32)
    xpw = np.asarray(inputs["x_proj_w"], np.float32)
    dtw = np.asarray(inputs["dt_head_w"], np.float32)
    dtb = np.asarray(inputs["dt_head_b"], np.float32)
    opw = np.asarray(inputs["out_proj_w"], np.float32)
    D = np.asarray(inputs["D"], np.float32)

    hT = np.ascontiguousarray(h[b].T).astype(BF)                        # [DM, L]
    wx = np.ascontiguousarray(
        ipw[ch].T.reshape(NK, 128, NG, 128).transpose(1, 0, 2, 3)
        .reshape(128, NK * NG * 128)).astype(BF)
    wz = np.ascontiguousarray(
        ipw[DI + j * DCH: DI + (j + 1) * DCH].T
        .reshape(NK, 128, NG, 128).transpose(1, 0, 2, 3)
        .reshape(128, NK * NG * 128)).astype(BF)
    wo = np.ascontiguousarray(
        opw[:, ch].T.reshape(NG, 128, NO, 128).transpose(1, 0, 2, 3)
        .reshape(128, NG * NO * 128)).astype(BF)
    dtwp = np.ascontiguousarray(dtw[ch].T.reshape(DTR, NG * 128)).astype(BF)
    xpwp = np.ascontiguousarray(
        xpw[:, ch].T.reshape(NG, 128, NR).transpose(1, 0, 2)
        .reshape(128, NG * NR)).astype(BF)

    # conv taps / D as diagonal matmul weights: cwd[p, g, j, m] = w_j[d] if
    # p == m else 0 (d = local channel g*128+p); dgd likewise with D.
    cwd = np.zeros((128, NG, 4, 128), np.float32)
    dgd = np.zeros((128, NG, 128), np.float32)
    pv = np.zeros((128, NG, 2), np.float32)
    r = np.arange(128)
    for g in range(NG):
        rows = slice(j * DCH + g * 128, j * DCH + (g + 1) * 128)
        cwd[r, g, :, r] = cw[rows]                  # [128, 4]
        dgd[r, g, r] = D[rows]
        pv[:, g, 0] = -dtb[rows]
        pv[:, g, 1] = cb[rows]

    return {
        "hT": hT,
        "wx": wx,
        "wz": wz,
        "wo": wo,
        "dtw": dtwp,
        "xpw": xpwp,
        "cwd": np.ascontiguousarray(cwd.reshape(128, NG * 4 * 128)).astype(BF),
        "dgd": np.ascontiguousarray(dgd.reshape(128, NG * 128)).astype(BF),
        "pvec": np.ascontiguousarray(pv.reshape(128, NG * 2)),
        "ones16": np.ones((NST, 128), np.float32).astype(BF),
        "id128": np.eye(128, dtype=np.float32).astype(BF),
    }


_CACHE = {}


def _get_module(key, *args, **kw):
    if key not in _CACHE:
        _CACHE[key] = build_module(*args, **kw)
    return _CACHE[key]


def run(inputs, trace=False, trace_cores=None):
    L, DM, DI = 2048, 1024, 2048
    DCH, NST, DTR = 512, 16, 64
    nc = _get_module("full", L, DM, DI, DCH, NST, DTR, 8, True)
    in_maps = []
    for core in range(8):
        b, j = divmod(core, 4)
        in_maps.append(pack_core_inputs(inputs, b, j, L, DM, DI, DCH, NST, DTR))
    res = run_bass_kernel_spmd(
        nc, in_maps, core_ids=list(range(8)), trace=trace,
        trace_cores=trace_cores)
    full = np.empty((2, L, DM), np.float32)
    for b in range(2):
        acc = res.results[4 * b]["outA"].astype(np.float64)
        acc = acc + res.results[4 * b]["outB"].astype(np.float64)
        for j in range(1, 4):
            acc = acc + res.results[4 * b + j]["outA"]
            acc = acc + res.results[4 * b + j]["outB"].astype(np.float64)
        full[b] = acc.T.astype(np.float32)
    return full, res


def kernel(**inputs) -> np.ndarray:
    out, _ = run(inputs, trace=False)
    return out


# revision 45
# speedup vs baseline: 1.1241x; 1.1241x over previous
"""Longhorn SSM layer on 8 Trainium2 cores.

Sharding: core (b, j) with b in {0,1}, j in {0..3} handles batch b and
d_inner channel chunk [j*512, (j+1)*512).  The x_proj contraction needs all
d_inner channels, so partial x_dbl results are AllReduced across the 4 cores
of each batch (split into two L-halves so the collective overlaps phase A).
The final out_proj partials are summed on the host.

v2 notes (vs v1):
  - all matmuls bf16 (fp32 matmuls ran at ~1/6 rate on PE)
  - depthwise conv as PE diagonal-weight matmuls accumulated in PSUM
  - gpsimd evicted from the scan phase (Pool shares an SBUF port with DVE
    and degraded DVE 2x-mode ops ~4x when running concurrently)
  - full-L scans (FD=2048, no inter-half carry), g-outer loop
  - D*x folded into the Y accumulation as a diag(D) matmul
  - z and y*gate stay in SBUF (no DRAM roundtrip)
"""

import numpy as np
import ml_dtypes

import concourse.bacc as bacc
import concourse.bass as bass
import concourse.tile as tile
from concourse import mybir
from concourse.bass_utils import run_bass_kernel_spmd

F32 = mybir.dt.float32
BF16 = mybir.dt.bfloat16
F16 = mybir.dt.float16
AL = mybir.AluOpType
AF = mybir.ActivationFunctionType

BF = ml_dtypes.bfloat16


def build_module(L, DM, DI, DCH, NST, DTR, num_devices, use_collective):
    NG = DCH // 128          # d-tiles per core (4)
    NK = DM // 128           # K-tiles for in_proj (8)
    NO = DM // 128           # out_proj output tiles (8)
    TQ = 512                 # matmul moving-dim tile
    NTQ = L // TQ            # 4
    LH = L // 2              # collective chunk (1024)
    NR = DTR + 2 * NST       # x_proj rows (96)
    PAD = 3                  # conv left pad

    nc = bacc.Bacc(
        "TRN2",
        target_bir_lowering=False,
        debug=False,
        enable_asserts=False,
        num_devices=num_devices,
    )

    # ---- I/O -------------------------------------------------------------
    hT_d = nc.dram_tensor("hT", [DM, L], BF16, kind="ExternalInput")
    wx_d = nc.dram_tensor("wx", [128, NK * NG * 128], BF16, kind="ExternalInput")
    wz_d = nc.dram_tensor("wz", [128, NK * NG * 128], BF16, kind="ExternalInput")
    wo_d = nc.dram_tensor("wo", [128, NG * NO * 128], BF16, kind="ExternalInput")
    dtw_d = nc.dram_tensor("dtw", [DTR, NG * 128], BF16, kind="ExternalInput")
    xpw_d = nc.dram_tensor("xpw", [128, NG * NR], BF16, kind="ExternalInput")
    cwd_d = nc.dram_tensor("cwd", [128, NG * 4 * 128], BF16, kind="ExternalInput")
    dgd_d = nc.dram_tensor("dgd", [128, NG * 128], BF16, kind="ExternalInput")
    pvec_d = nc.dram_tensor("pvec", [128, NG * 2], F32, kind="ExternalInput")
    ones_d = nc.dram_tensor("ones16", [NST, 128], BF16, kind="ExternalInput")
    id_d = nc.dram_tensor("id128", [128, 128], BF16, kind="ExternalInput")
    outA_d = nc.dram_tensor("outA", [DM, L], F32, kind="ExternalOutput")
    outB_d = nc.dram_tensor("outB", [DM, L], BF16, kind="ExternalOutput")

    # internal DRAM
    cc_in = [nc.dram_tensor(f"ccin{h}", [NR, LH], BF16, kind="Internal")
             for h in range(2)]
    cc_out = [nc.dram_tensor(f"ccout{h}", [NR, LH], BF16, kind="Internal")
              for h in range(2)]
    kbd = nc.dram_tensor("kbd", [NST, L], BF16, kind="Internal")
    qbd = nc.dram_tensor("qbd", [NST, L], BF16, kind="Internal")
    kkbd = nc.dram_tensor("kkbd", [NST, L], BF16, kind="Internal")

    groups = [[0, 1, 2, 3], [4, 5, 6, 7]] if num_devices == 8 else [[0]]

    with tile.TileContext(nc) as tc:
        with (
            tc.tile_pool(name="const", bufs=1) as constp,
            tc.tile_pool(name="persist", bufs=1) as pp,
        ):
            ones_sb = constp.tile([NST, 128], BF16)
            nc.sync.dma_start(ones_sb, ones_d.ap())
            id_sb = constp.tile([128, 128], BF16)
            nc.sync.dma_start(id_sb, id_d.ap())
            dgd_sb = constp.tile([128, NG, 128], BF16)
            nc.sync.dma_start(dgd_sb, dgd_d.ap().rearrange("p (g m) -> p g m", g=NG))
            pvec = constp.tile([128, NG, 2], F32)   # [...,0]=-dtb, [...,1]=conv_b
            nc.sync.dma_start(pvec, pvec_d.ap().rearrange("p (g c) -> p g c", g=NG))

            # persistent SBUF through the scan phase (bf16, 2 bytes)
            xs = pp.tile([128, NG, L], BF16)      # silu(conv(x))
            zs2p = pp.tile([128, NG, L], BF16)    # silu(z) (gate)
            dtvb = pp.tile([128, NG, L], BF16)    # dtv
            ub = pp.tile([128, NG, L], BF16)      # xs*dtv
            ygb = pp.tile([128, NG, L], BF16)     # (y + D*xs)*silu(z)

            # ------- phases A (x-side in_proj/conv/x_dbl + CC), A2, z -----
            # One pool scope: A2's PSUM tiles share slots with phase-A tags
            # (same shapes), and the z-side matmuls reuse hsbs/wz without a
            # re-DMA.  Emission order sets scheduler priority:
            #   x-side+CC | A2(h0) | z(tq0,1) | A2(h1) | z(tq2,3)
            with (
                tc.tile_pool(name="hw", bufs=1) as hwp,
                tc.tile_pool(name="xpre", bufs=1) as xprep,
                tc.tile_pool(name="psA", bufs=3, space="PSUM") as psA,
                tc.tile_pool(name="psC", bufs=2, space="PSUM") as psCp,
                tc.tile_pool(name="psX", bufs=1, space="PSUM") as psXp,
                tc.tile_pool(name="asm", bufs=3) as asmp,
                tc.tile_pool(name="dtw", bufs=1) as dtwp,
                tc.tile_pool(name="rows", bufs=2) as rowp,
                tc.tile_pool(name="dtv", bufs=3) as dtvp,
            ):
                # DMA priority: first-tq activations + x-weights first
                wx_sb = hwp.tile([128, NK, NG, 128], BF16)
                hsbs = []
                for tq in range(NTQ):
                    hsbs.append(hwp.tile([128, NK, TQ], BF16, name=f"hsb{tq}",
                                         tag=f"hsb{tq}"))
                for k in range(NK):
                    nc.sync.dma_start(
                        hsbs[0][:, k], hT_d.ap()[k * 128:(k + 1) * 128, 0:TQ])
                    nc.sync.dma_start(
                        wx_sb[:, k], wx_d.ap()[:, k * NG * 128:(k + 1) * NG * 128]
                        .rearrange("p (g m) -> p g m", g=NG))
                cw_sb = hwp.tile([128, NG, 4, 128], BF16)
                nc.sync.dma_start(
                    cw_sb, cwd_d.ap().rearrange("p (g j m) -> p g j m", g=NG, j=4))
                xpw_sb = hwp.tile([128, NG, NR], BF16)
                for g in range(NG):
                    nc.sync.dma_start(
                        xpw_sb[:, g], xpw_d.ap()[:, g * NR:(g + 1) * NR])
                for tq in range(1, NTQ):
                    ts = slice(tq * TQ, (tq + 1) * TQ)
                    for k in range(NK):
                        nc.sync.dma_start(
                            hsbs[tq][:, k], hT_d.ap()[k * 128:(k + 1) * 128, ts])
                wz_sb = hwp.tile([128, NK, NG, 128], BF16)
                for k in range(NK):
                    nc.sync.dma_start(
                        wz_sb[:, k], wz_d.ap()[:, k * NG * 128:(k + 1) * NG * 128]
                        .rearrange("p (g m) -> p g m", g=NG))
                dtw_sb = dtwp.tile([DTR, NG, 128], BF16)
                nc.sync.dma_start(
                    dtw_sb, dtw_d.ap().rearrange("p (g m) -> p g m", g=NG))

                xpre = xprep.tile([128, NG, L + PAD], BF16)
                for g in range(NG):
                    nc.vector.memset(xpre[:, g, 0:PAD], 0.0)

                # ---- x-side in_proj + conv + x_dbl + collective ----------
                for h in range(2):
                    for tq2 in range(2):
                        tq = 2 * h + tq2
                        ts = slice(tq * TQ, (tq + 1) * TQ)
                        for g in range(NG):
                            ps = psA.tile([128, TQ], F32, name="ps_xz",
                                          tag="psxz")
                            for k in range(NK):
                                nc.tensor.matmul(ps, wx_sb[:, k, g, :],
                                                 hsbs[tq][:, k, :],
                                                 start=(k == 0),
                                                 stop=(k == NK - 1))
                            nc.scalar.copy(
                                xpre[:, g, PAD + tq * TQ: PAD + (tq + 1) * TQ],
                                ps)
                            pc = psCp.tile([128, TQ], F32, name="pc", tag="pc")
                            for j in range(4):
                                nc.tensor.matmul(
                                    pc, cw_sb[:, g, j, :],
                                    xpre[:, g, tq * TQ + j: tq * TQ + j + TQ],
                                    start=(j == 0), stop=(j == 3))
                            nc.scalar.activation(xs[:, g, ts], pc, AF.Silu,
                                                 bias=pvec[:, g, 1:2])
                    psX = psXp.tile([NR, LH], F32, name="psX", tag="psX")
                    for tq2 in range(2):
                        for g in range(NG):
                            ss = slice(tq2 * TQ, (tq2 + 1) * TQ)
                            nc.tensor.matmul(
                                psX[:, ss], xpw_sb[:, g, :],
                                xs[:, g, h * LH + tq2 * TQ:
                                   h * LH + (tq2 + 1) * TQ],
                                start=(g == 0), stop=(g == NG - 1))
                    xdp = asmp.tile([NR, LH], BF16, name="xdp", tag="xdp",
                                    bufs=2)
                    nc.scalar.copy(xdp, psX)
                    nc.sync.dma_start(cc_in[h].ap(), xdp)
                    if use_collective:
                        nc.gpsimd.collective_compute(
                            "AllReduce", AL.add, replica_groups=groups,
                            ins=[cc_in[h].ap()], outs=[cc_out[h].ap()])
                    else:
                        nc.sync.dma_start(cc_out[h].ap(), cc_in[h].ap())

                # ---- z-side in_proj: fills the PE idle in the CC shadows;
                # finishing early also releases hsbs so phase B's SBUF pools
                # can allocate before the scans need them.  zs2 = silu(z)
                # computed straight from PSUM.
                for tq in range(NTQ):
                    ts = slice(tq * TQ, (tq + 1) * TQ)
                    for g in range(NG):
                        psz = psA.tile([128, TQ], F32, name="ps_z",
                                       tag="psxz")
                        for k in range(NK):
                            nc.tensor.matmul(psz, wz_sb[:, k, g, :],
                                             hsbs[tq][:, k, :],
                                             start=(k == 0),
                                             stop=(k == NK - 1))
                        nc.scalar.activation(zs2p[:, g, ts], psz, AF.Silu)

                # ---- A2 per half ----------------------------------------

                for h in range(2):
                    hs = slice(h * LH, (h + 1) * LH)
                    # stage k/q rows for full-L broadcasts: DRAM -> DRAM
                    nc.sync.dma_start(kbd.ap()[:, hs],
                                      cc_out[h].ap()[DTR:DTR + NST, :])
                    nc.sync.dma_start(qbd.ap()[:, hs],
                                      cc_out[h].ap()[DTR + NST:NR, :])
                    dtl = rowp.tile([DTR, LH], BF16, name="dtl", tag="dtl")
                    nc.sync.dma_start(dtl, cc_out[h].ap()[0:DTR, :])
                    krow = rowp.tile([NST, LH], BF16, name="krow", tag="krow")
                    nc.sync.dma_start(krow, cc_out[h].ap()[DTR:DTR + NST, :])
                    kk = rowp.tile([NST, LH], F32, name="kk", tag="kk")
                    nc.scalar.activation(kk, krow, AF.Square)
                    kkb16 = rowp.tile([NST, LH], BF16, name="kkb16", tag="kkb16")
                    nc.scalar.copy(kkb16, kk)
                    nc.sync.dma_start(kkbd.ap()[:, hs], kkb16)
                    # SK[t] = sum_n kk broadcast to 128 partitions
                    psKs = rowp.tile([128, LH], BF16, name="psKs", tag="psKs")
                    for s2 in range(2):
                        ss = slice(s2 * TQ, (s2 + 1) * TQ)
                        psK = psA.tile([128, TQ], F32, name="psK", tag="psxz")
                        nc.tensor.matmul(psK, ones_sb, kkb16[:, ss],
                                         start=True, stop=True)
                        nc.scalar.copy(psKs[:, ss], psK)
                    # dtv = 1/(1 + E + SK) = sigmoid(-ln(E + SK)).
                    # g0 gets a dedicated minimum-latency chain (its dtv/ub
                    # gate the first scans); g1..3 batch per ACT function.
                    def dtv_chain(gl):
                        egs, dens, lnws = [], [], []
                        for g in gl:
                            eg = dtvp.tile([128, LH], BF16, name=f"eg{g}",
                                           tag=f"eg{g}", bufs=1)
                            for s2 in range(2):
                                ss = slice(s2 * TQ, (s2 + 1) * TQ)
                                psD = psCp.tile([128, TQ], F32, name="psD",
                                                tag="pc")
                                nc.tensor.matmul(psD, dtw_sb[:, g, :],
                                                 dtl[:, ss], start=True,
                                                 stop=True)
                                nc.scalar.activation(eg[:, ss], psD, AF.Exp,
                                                     bias=pvec[:, g, 0:1],
                                                     scale=-1.0)
                            egs.append(eg)
                        for g, eg in zip(gl, egs):
                            den = dtvp.tile([128, LH], BF16, name=f"den{g}",
                                            tag=f"den{g}", bufs=1)
                            nc.vector.tensor_tensor(den, eg, psKs, op=AL.add)
                            dens.append(den)
                        for g, den in zip(gl, dens):
                            lnw = dtvp.tile([128, LH], BF16, name=f"lnw{g}",
                                            tag=f"lnw{g}", bufs=1)
                            nc.scalar.activation(lnw, den, AF.Ln)
                            lnws.append(lnw)
                        for g, lnw in zip(gl, lnws):
                            nc.scalar.activation(dtvb[:, g, hs], lnw,
                                                 AF.Sigmoid, scale=-1.0)
                            nc.vector.tensor_tensor(ub[:, g, hs], xs[:, g, hs],
                                                    dtvb[:, g, hs], op=AL.mult)
                    dtv_chain([0])
                    dtv_chain([1, 2, 3])

            # ---------------- phase B: the scan + out_proj ----------------
            # out_proj is split: outA = g0+g1+g2 contributions, computed and
            # DMA'd while g3's scans keep the DVE busy (psY bufs=1 leaves 4
            # PSUM banks for it); outB = g3's contribution in the tail with
            # its PSUM drain on the then-idle DVE.  The host adds outA+outB.
            with (
                tc.tile_pool(name="psY", bufs=1, space="PSUM") as psYp,
                tc.tile_pool(name="psO", bufs=4, space="PSUM") as psOp,
                tc.tile_pool(name="wo", bufs=1) as wop,
                tc.tile_pool(name="bcast", bufs=3) as bcp,
                tc.tile_pool(name="scan", bufs=2) as scp,
                tc.tile_pool(name="odr", bufs=3) as odp,
            ):
                wo_sb = wop.tile([128, NG, NO, 128], BF16)
                for g in range(NG):
                    if g == 1:
                        # wo load queued after g0's broadcasts so it doesn't
                        # delay the first scans
                        for g2 in range(NG):
                            nc.sync.dma_start(
                                wo_sb[:, g2],
                                wo_d.ap()[:, g2 * NO * 128:(g2 + 1) * NO * 128]
                                .rearrange("p (o m) -> p o m", o=NO))
                    Y = psYp.tile([128, L], F32, name="Y", tag="Y")
                    for n in range(NST):
                        kkb_t = bcp.tile([128, L], BF16, name="kkb_t", tag="kkb")
                        nc.sync.dma_start(
                            kkb_t, kkbd.ap()[n:n + 1, :].broadcast_to([128, L]))
                        kb_t = bcp.tile([128, L], BF16, name="kb_t", tag="kb")
                        nc.sync.dma_start(
                            kb_t, kbd.ap()[n:n + 1, :].broadcast_to([128, L]))
                        qb_t = bcp.tile([128, L], BF16, name="qb_t", tag="qb",
                                        bufs=2)
                        nc.sync.dma_start(
                            qb_t, qbd.ap()[n:n + 1, :].broadcast_to([128, L]))
                        c_t = scp.tile([128, L], BF16, name="c_t", tag="c")
                        nc.vector.tensor_tensor(c_t, dtvb[:, g, :], kkb_t,
                                                op=AL.mult)
                        a_t = scp.tile([128, L], F16, name="a_t", tag="a")
                        nc.scalar.activation(a_t, c_t, AF.Identity,
                                             bias=1.0, scale=-1.0)
                        b_t = scp.tile([128, L], BF16, name="b_t", tag="b")
                        nc.vector.tensor_tensor(b_t, ub[:, g, :], kb_t,
                                                op=AL.mult)
                        s_t = scp.tile([128, L], BF16, name="s_t", tag="s",
                                       bufs=3)
                        nc.vector.tensor_tensor_scan(
                            s_t, a_t, b_t, 0.0, op0=AL.mult, op1=AL.add)
                        p_t = scp.tile([128, L], BF16, name="p_t", tag="p",
                                       bufs=4)
                        nc.vector.tensor_tensor(p_t, s_t, qb_t, op=AL.mult)
                        for h4 in range(4):
                            nc.tensor.matmul(
                                Y[:, h4 * TQ:(h4 + 1) * TQ],
                                id_sb, p_t[:, h4 * TQ:(h4 + 1) * TQ],
                                start=(n == 0), stop=False)
                    # skip term D*xs folded into the PSUM accumulation
                    for h4 in range(4):
                        nc.tensor.matmul(
                            Y[:, h4 * TQ:(h4 + 1) * TQ],
                            dgd_sb[:, g, :],
                            xs[:, g, h4 * TQ:(h4 + 1) * TQ],
                            start=False, stop=True)
                    # drain: ygb = (y + D*xs) * silu(z)
                    nc.vector.tensor_tensor(ygb[:, g, :], Y, zs2p[:, g, :],
                                            op=AL.mult)
                # outA = out_proj over g0..g2.  Emitted after the scan loop so
                # the scheduler only slots it into PE/ACT idle time during
                # g3's scans.  Full [128, L] rows: one DMA per o (the per-tile
                # DIRECT2D issue on the Sync engine was the tail bottleneck).
                for o in range(NO):
                    ot = odp.tile([128, L], F32, name="ot", tag="ot", bufs=1)
                    for tq in range(NTQ):
                        ts = slice(tq * TQ, (tq + 1) * TQ)
                        po = psOp.tile([128, TQ], F32, name="po", tag="po")
                        for g2 in range(3):
                            nc.tensor.matmul(po, wo_sb[:, g2, o, :],
                                             ygb[:, g2, ts],
                                             start=(g2 == 0), stop=(g2 == 2))
                        nc.scalar.copy(ot[:, ts], po)
                    nc.sync.dma_start(outA_d.ap()[o * 128:(o + 1) * 128, :], ot)
                # outB = g3's contribution (tail): bf16, PSUM drains split
                # across the then-idle DVE and ACT
                for o in range(NO):
                    ot = odp.tile([128, L], BF16, name="otb", tag="otb", bufs=2)
                    for tq in range(NTQ):
                        ts = slice(tq * TQ, (tq + 1) * TQ)
                        po = psOp.tile([128, TQ], F32, name="po", tag="po")
                        nc.tensor.matmul(po, wo_sb[:, 3, o, :], ygb[:, 3, ts],
                                         start=True, stop=True)
                        if tq % 2 == 0:
                            nc.vector.tensor_copy(ot[:, ts], po)
                        else:
                            nc.scalar.copy(ot[:, ts], po)
                    nc.sync.dma_start(outB_d.ap()[o * 128:(o + 1) * 128, :], ot)

    nc.compile()
    return nc


# ----------------------------------------------------------------------------
# host-side packing
# ----------------------------------------------------------------------------

def pack_core_inputs(inputs, b, j, L, DM, DI, DCH, NST, DTR):
    NG = DCH // 128
    NK = DM // 128
    NO = DM // 128
    NR = DTR + 2 * NST
    ch = slice(j * DCH, (j + 1) * DCH)

    h = np.asarray(inputs["hidden_states"], np.float32)
    ipw = np.asarray(inputs["in_proj_w"], np.float32)
    cw = np.asarray(inputs["conv_w"], np.float32).reshape(DI, 4)
    cb = np.asarray(inputs["conv_b"], np.float32)
    xpw = np.asarray(inputs["x_proj_w"], np.float32)
    dtw = np.asarray(inputs["dt_head_w"], np.float32)
    dtb = np.asarray(inputs["dt_head_b"], np.float32)
    opw = np.asarray(inputs["out_proj_w"], np.float32)
    D = np.asarray(inputs["D"], np.float32)

    hT = np.ascontiguousarray(h[b].T).astype(BF)                        # [DM, L]
    wx = np.ascontiguousarray(
        ipw[ch].T.reshape(NK, 128, NG, 128).transpose(1, 0, 2, 3)
        .reshape(128, NK * NG * 128)).astype(BF)
    wz = np.ascontiguousarray(
        ipw[DI + j * DCH: DI + (j + 1) * DCH].T
        .reshape(NK, 128, NG, 128).transpose(1, 0, 2, 3)
        .reshape(128, NK * NG * 128)).astype(BF)
    wo = np.ascontiguousarray(
        opw[:, ch].T.reshape(NG, 128, NO, 128).transpose(1, 0, 2, 3)
        .reshape(128, NG * NO * 128)).astype(BF)
    dtwp = np.ascontiguousarray(dtw[ch].T.reshape(DTR, NG * 128)).astype(BF)
    xpwp = np.ascontiguousarray(
        xpw[:, ch].T.reshape(NG, 128, NR).transpose(1, 0, 2)
        .reshape(128, NG * NR)).astype(BF)

    # conv taps / D as diagonal matmul weights: cwd[p, g, j, m] = w_j[d] if
    # p == m else 0 (d = local channel g*128+p); dgd likewise with D.
    cwd = np.zeros((128, NG, 4, 128), np.float32)
    dgd = np.zeros((128, NG, 128), np.float32)
    pv = np.zeros((128, NG, 2), np.float32)
    r = np.arange(128)
    for g in range(NG):
        rows = slice(j * DCH + g * 128, j * DCH + (g + 1) * 128)
        cwd[r, g, :, r] = cw[rows]                  # [128, 4]
        dgd[r, g, r] = D[rows]
        pv[:, g, 0] = -dtb[rows]
        pv[:, g, 1] = cb[rows]

    return {
        "hT": hT,
        "wx": wx,
        "wz": wz,
        "wo": wo,
        "dtw": dtwp,
        "xpw": xpwp,
        "cwd": np.ascontiguousarray(cwd.reshape(128, NG * 4 * 128)).astype(BF),
        "dgd": np.ascontiguousarray(dgd.reshape(128, NG * 128)).astype(BF),
        "pvec": np.ascontiguousarray(pv.reshape(128, NG * 2)),
        "ones16": np.ones((NST, 128), np.float32).astype(BF),
        "id128": np.eye(128, dtype=np.float32).astype(BF),
    }


_CACHE = {}


def _get_module(key, *args, **kw):
    if key not in _CACHE:
        _CACHE[key] = build_module(*args, **kw)
    return _CACHE[key]


def run(inputs, trace=False, trace_cores=None):
    L, DM, DI = 2048, 1024, 2048
    DCH, NST, DTR = 512, 16, 64
    nc = _get_module("full", L, DM, DI, DCH, NST, DTR, 8, True)
    in_maps = []
    for core in range(8):
        b, j = divmod(core, 4)
        in_maps.append(pack_core_inputs(inputs, b, j, L, DM, DI, DCH, NST, DTR))
    res = run_bass_kernel_spmd(
        nc, in_maps, core_ids=list(range(8)), trace=trace,
        trace_cores=trace_cores)
    full = np.empty((2, L, DM), np.float32)
    for b in range(2):
        acc = res.results[4 * b]["outA"].astype(np.float64)
        acc = acc + res.results[4 * b]["outB"].astype(np.float64)
        for j in range(1, 4):
            acc = acc + res.results[4 * b + j]["outA"]
            acc = acc + res.results[4 * b + j]["outB"].astype(np.float64)
        full[b] = acc.T.astype(np.float32)
    return full, res


def kernel(**inputs) -> np.ndarray:
    out, _ = run(inputs, trace=False)
    return out

